# revision 1
# baseline (speedup 1.0000x reference)
"""TRN2 Bass kernel for nn_BSLinear_71159018160311.

Computes  out = input @ W.T  with
  W = U @ diag(weight^2 * mask) @ Vh + U_additional @ Vh_additional

Sharding: data-parallel over the B*S=16384 token dim across 8 NeuronCores
(2048 tokens/core), no collectives. Each core runs the factorized form
(t = V_eff @ x, then y = U_eff @ t) as two matmul phases.

Mixed-precision rank split: the rank-r component of W contributes with
weight s_r = weight_r^2 * mask_r, so fp8-ing a rank subset with s^2-energy
share E costs rel err ~ beta*sqrt(2E) per phase (beta ~ 2.7% for e4m3 on
Gaussian data). The two phases have independent budgets: phase 1 runs the
640 lowest-s ranks in fp8 e4m3 DoubleRow (2 k-tiles contracted per PE
pass - 2x rate), phase 2 the lowest 512 (rank-pairs must be even); the
128 mid ranks are cast from the fp8 psum to true-scale bf16 and join the
bf16 side of phase 2. Everything else runs bf16 (full PE rate, ~0.1%
error). Measured end-to-end rel err 1.60e-2 vs the 2e-2 gate.

Scales (all powers of two, so rescaling is exact and program immediates are
data-independent): x*32 -> fp8, V*2048 -> fp8, U_eff*4096 -> fp8, and the
phase-1 psum (scale 32*2048) is cast to the phase-2 fp8 operand t8 with a
single tensor_scalar_mul by 2^-12 (target scale 16). The bf16 branch's U is
pre-scaled by C=16*4096=2^16 so both branches accumulate in the SAME psum
group; the output copy multiplies by 2^-16.

Phase 1, bf16 half: token-halved full-K psum - all (<=4) bf16 rank-tile
psum groups ([128,1024], 2 banks each) stay open across the whole K=4096
contraction for one token-half, so there are no SBUF accumulation adds at
all; the second token-half reuses the x ring slots. The fp8 half keeps x8
(8MB) + v8 resident in SBUF (their loads queue behind both x token-half
streams and land just-in-time), accumulates full-K in psum, then one
scaled cast per half-group emits t8 in the [128, 2, TC] k-pair layout
DoubleRow wants (or true-scale bf16 for the mid tiles). Phase 2
accumulates bf16 rank-tiles then fp8 rank-pairs into one psum group per
128-row dout tile (u chunks prefetched one 512-dout chunk ahead of the
y-store DMAs), scaled-copies to SBUF on alternating DVE/ACT and DMAs out;
the very last group is split 1024/512/512 to shorten the drain tail.

When U_additional/Vh_additional are nonzero (they are zero for this
problem instance) the A=64 extra ranks join the bf16 half (zero-padded to
a full 128 tile).
"""

import functools

import numpy as np

B, S, D_IN, D_OUT, R, A = 4, 4096, 4096, 4096, 1024, 64
N_CORES = 8
T = B * S
TC = T // N_CORES  # 2048
KT = D_IN // 128  # 32 k-tiles
KP = KT // 2  # 16 k-pairs (fp8 DoubleRow)
KB = 4  # bf16 k-tiles per stream block
NB = KT // KB  # 8 blocks
NC_OUT = D_OUT // 512  # 8 chunks of 512 dout rows

# power-of-two scales (exact rescaling, data-independent immediates)
AX, AV, AU, AT = 32.0, 2048.0, 4096.0, 16.0
C_SCALE = AT * AU  # 65536
C1 = AT / (AX * AV)  # 2^-12: psum(phase1 fp8) -> t8
INV_C = 1.0 / C_SCALE  # 2^-16: psum(phase2) -> y


@functools.lru_cache(maxsize=4)
def _build(NBF1, NS81, NS82, devcast=False):
    """NBF1: phase-1 bf16 rank-tiles; NS81: phase-1 fp8 tiles; NS82: phase-2
    fp8 tiles (even, subset of the phase-1 fp8 set). Ranks are laid out in
    s-ascending order; tiles [NS82, NS81) are fp8 in phase 1 but bf16 in
    phase 2 (their t is cast to bf16 at true scale)."""
    import concourse.bacc as bacc
    import concourse.mybir as mybir
    import concourse.tile as tile

    NP8 = NS82 // 2  # phase-2 fp8 rank-pair tiles
    NMID = NS81 - NS82  # fp8-phase1 / bf16-phase2 tiles
    NBF2 = NBF1 + NMID  # phase-2 bf16 rank-tiles
    NF = NBF1 * 128
    NS = NS81 * 128
    f8 = mybir.dt.float8e4
    bf16 = mybir.dt.bfloat16
    f32 = mybir.dt.float32
    add = mybir.AluOpType.add
    ACT_COPY = mybir.ActivationFunctionType.Copy
    DR = mybir.MatmulPerfMode.DoubleRow
    H = TC // 2  # 1024: psum half-group token width
    CMID = 1.0 / (AX * AV)  # 2^-16: phase-1 fp8 psum -> true-scale bf16 t

    nc = bacc.Bacc(trn_type="TRN2")
    with tile.TileContext(nc) as tc:
        with tc.tile_pool(name="dram", bufs=1, space="DRAM") as dram:
            xbf = dram.tile([KT, 128, TC], bf16, kind="ExternalInput", name="xbf")
            vbf = dram.tile([KT, 128, NF], bf16, kind="ExternalInput", name="vbf")
            ubf = dram.tile([128, NBF2, D_OUT], bf16, kind="ExternalInput", name="ubf")
            if NS81:
                if not devcast:
                    x8d = dram.tile(
                        [KP, 128, 2, TC], f8, kind="ExternalInput", name="x8"
                    )
                v8d = dram.tile([KP, 128, 2, NS], f8, kind="ExternalInput", name="v8")
                u8d = dram.tile(
                    [128, NP8, 2, D_OUT], f8, kind="ExternalInput", name="u8"
                )
            yT = dram.tile([D_OUT, TC], f32, kind="ExternalOutput", name="yT")

            with (
                tc.tile_pool(name="tbf", bufs=NBF2) as tbfpool,
                tc.tile_pool(name="t8", bufs=max(NP8, 1)) as t8pool,
                tc.tile_pool(name="x8r", bufs=max(KP, 1)) as x8pool,
                tc.tile_pool(name="v8r", bufs=max(KP, 1)) as v8pool,
                tc.tile_pool(name="u0", bufs=1) as u0pool,
                tc.tile_pool(name="ps", bufs=4, space="PSUM") as pspool,
            ):
                t_bf = [tbfpool.tile([128, TC], bf16, name="tbf") for _ in range(NBF2)]
                t8 = [t8pool.tile([128, 2, TC], f8, name="t8") for _ in range(NP8)]
                x8_t = [
                    x8pool.tile([128, 2, TC], f8, name="x8r")
                    for _ in range(KP if NS81 else 0)
                ]
                v8_t = [
                    v8pool.tile([128, 2, NS], f8, name="v8r")
                    for _ in range(KP if NS81 else 0)
                ]
                # phase-2 chunk-0 weights: loaded in background during phase 1
                u0bf = u0pool.tile([128, NBF2, 512], bf16)
                u08 = u0pool.tile([128, max(NP8, 1), 2, 512], f8)

                # ---- phase 1: bf16 half (token-halved full-K psum) ----
                # all NBF1<=4 rank-tile psum groups (2 banks each) stay open
                # across the whole K=4096 contraction for one token-half, so
                # the bf16 half needs NO SBUF accumulation adds at all; the
                # second token-half reuses the x ring slots (WAR-linked).
                with (
                    tc.tile_pool(name="xk", bufs=KT) as xpool,
                    tc.tile_pool(name="vk", bufs=KT) as vpool,
                ):
                    vts = [
                        vpool.tile([128, NF], bf16, name="vk") for _ in range(KT)
                    ]
                    MCH = min(NBF1, 4)
                    passes = [
                        (list(range(c, min(c + MCH, NBF1))), h)
                        for c in range(0, NBF1, MCH)
                        for h in range(2)
                    ]

                    def issue_pass(pi):
                        ch, h = passes[pi]
                        xts = []
                        for k in range(KT):
                            t = xpool.tile([128, H], bf16, name="xk")
                            if pi == 0:
                                nc.sync.dma_start(vts[k][:], vbf[k])
                            if pi == 0 and k == 0:
                                # startup: 512-col slices so the first matmul
                                # fires as soon as slice 0 lands
                                for q in range(2):
                                    nc.sync.dma_start(
                                        t[:, q * 512 : (q + 1) * 512],
                                        xbf[0][:, q * 512 : (q + 1) * 512],
                                    )
                            else:
                                nc.sync.dma_start(
                                    t[:], xbf[k][:, h * H : (h + 1) * H]
                                )
                            xts.append(t)
                        return xts

                    pending = {0: issue_pass(0)}
                    if len(passes) > 1:
                        pending[1] = issue_pass(1)
                    # resident fp8 operands + phase-2 chunk-0 weights queue
                    # behind both token-half streams; the fp8 phase consumes
                    # pairs in order just as they land
                    if NS81:
                        for p in range(KP):
                            nc.sync.dma_start(x8_t[p][:], x8d[p])
                            nc.sync.dma_start(v8_t[p][:], v8d[p])
                    nc.sync.dma_start(u0bf[:], ubf[:, :, 0:512])
                    if NS82:
                        nc.sync.dma_start(u08[:], u8d[:, :, :, 0:512])

                    for pi, (ch, h) in enumerate(passes):
                        xts = pending.pop(pi)
                        if pi + 2 < len(passes) and pi + 2 not in pending:
                            pending[pi + 2] = issue_pass(pi + 2)
                        psums = [
                            pspool.tile([128, H], f32, name="ps") for _ in ch
                        ]
                        for k in range(KT):
                            for mi, m in enumerate(ch):
                                for n2 in range(2):
                                    nc.tensor.matmul(
                                        psums[mi][:, n2 * 512 : (n2 + 1) * 512],
                                        lhsT=vts[k][:, m * 128 : (m + 1) * 128],
                                        rhs=xts[k][:, n2 * 512 : (n2 + 1) * 512],
                                        start=(k == 0),
                                        stop=(k == KT - 1),
                                    )
                        for mi, m in enumerate(ch):
                            nc.any.tensor_copy(
                                t_bf[NMID + m][:, h * H : (h + 1) * H],
                                psums[mi][:, :],
                            )

                # ---- phase 1: fp8 half (x8 resident, full-K psum) ----
                # mid tiles (bf16 in phase 2) first: phase-2's first bf16
                # matmuls read them, so their casts must land earliest
                for m8 in list(range(NS82, NS81)) + list(range(NS82)):
                    for h in range(2):
                        psum = pspool.tile([128, H], f32, name="ps")
                        for p in range(KP):
                            for ni in range(2):
                                n = 2 * h + ni
                                nc.tensor.matmul(
                                    psum[:, ni * 512 : (ni + 1) * 512],
                                    lhsT=v8_t[p][:, :, m8 * 128 : (m8 + 1) * 128],
                                    rhs=x8_t[p][:, :, n * 512 : (n + 1) * 512],
                                    start=(p == 0),
                                    stop=(p == KP - 1),
                                    perf_mode=DR,
                                )
                        if m8 < NS82:
                            nc.any.tensor_scalar_mul(
                                t8[m8 // 2][:, m8 % 2, h * H : (h + 1) * H],
                                psum[:, :],
                                C1,
                            )
                        else:
                            nc.any.tensor_scalar_mul(
                                t_bf[m8 - NS82][:, h * H : (h + 1) * H],
                                psum[:, :],
                                CMID,
                            )

                # ---- phase 2 ----
                with (
                    tc.tile_pool(name="ud", bufs=2) as upool,
                    tc.tile_pool(name="ysb", bufs=4) as ypool,
                ):
                    u_cur = (u0bf, u08)
                    u_next = None
                    for ci in range(NC_OUT):
                        ubf_t, u8_t = u_cur
                        if ci + 1 < NC_OUT:
                            # prefetch next chunk's weights ahead of this
                            # chunk's y-store DMAs so the PE never waits
                            nci = ci + 1
                            nbf = upool.tile([128, NBF2, 512], bf16, name="udb")
                            nc.sync.dma_start(
                                nbf[:], ubf[:, :, nci * 512 : (nci + 1) * 512]
                            )
                            n8 = None
                            if NS82:
                                n8 = upool.tile([128, NP8, 2, 512], f8, name="ud8")
                                nc.sync.dma_start(
                                    n8[:], u8d[:, :, :, nci * 512 : (nci + 1) * 512]
                                )
                            u_next = (nbf, n8)
                        for dd in range(4):
                            row = ci * 512 + dd * 128
                            # the very last half-group is split into 512-wide
                            # quarters so the post-matmul copy+store tail is
                            # as short as possible
                            final = ci == NC_OUT - 1 and dd == 3
                            pieces = [(0, (0, 1)), (1, (2,)), (2, (3,))] if final else [
                                (0, (0, 1)), (1, (2, 3))
                            ]
                            for pi, (slot, ns_) in enumerate(pieces):
                                W = len(ns_) * 512
                                psum = pspool.tile([128, W], f32, name="ps")
                                for kt in range(NBF2):
                                    for ni, n in enumerate(ns_):
                                        nc.tensor.matmul(
                                            psum[:, ni * 512 : (ni + 1) * 512],
                                            lhsT=ubf_t[:, kt, dd * 128 : (dd + 1) * 128],
                                            rhs=t_bf[kt][:, n * 512 : (n + 1) * 512],
                                            start=(kt == 0),
                                            stop=(NS82 == 0 and kt == NBF2 - 1),
                                        )
                                for pt in range(NP8):
                                    for ni, n in enumerate(ns_):
                                        nc.tensor.matmul(
                                            psum[:, ni * 512 : (ni + 1) * 512],
                                            lhsT=u8_t[:, pt, :, dd * 128 : (dd + 1) * 128],
                                            rhs=t8[pt][:, :, n * 512 : (n + 1) * 512],
                                            start=False,
                                            stop=(pt == NP8 - 1),
                                            perf_mode=DR,
                                        )
                                ysb = ypool.tile([128, W], f32, name="ysb")
                                if pi == 1:
                                    # ACT: out = Copy(in * scale); GpSimd
                                    # cannot read PSUM on hardware
                                    nc.scalar.activation(
                                        ysb[:], psum[:, :], ACT_COPY, scale=INV_C
                                    )
                                else:
                                    nc.vector.tensor_scalar_mul(
                                        ysb[:], psum[:, :], INV_C
                                    )
                                col = ns_[0] * 512
                                nc.sync.dma_start(
                                    yT[row : row + 128, col : col + W],
                                    ysb[:],
                                )
                        u_cur = u_next
    nc.compile()
    names = {"xbf": xbf.name, "vbf": vbf.name, "ubf": ubf.name, "yT": yT.name}
    if NS81:
        names.update({"v8": v8d.name, "u8": u8d.name})
        if not devcast:
            names["x8"] = x8d.name
    return nc, names


def _select(weight, mask, U_additional, Vh_additional):
    """Pick (NBF1, NS81, NS82) from the weights: the largest fp8 coverage
    whose estimated error keeps margin inside the 2e-2 gate. Phase-1 fp8
    covers tiles [0, NS81), phase-2 fp8 tiles [0, NS82) (even), ranks laid
    out s-ascending."""
    s = (weight * weight * mask).astype(np.float32)
    order = np.argsort(s, kind="stable")
    s2 = np.sort(s.astype(np.float64) ** 2)
    tot = float(np.sum(s2)) or 1.0
    cum = np.cumsum(s2) / tot

    def E(tiles):
        n = tiles * 128
        return float(cum[n - 1]) if n else 0.0

    BETA, BFERR, THR = 0.027, 4.5e-3, 1.65e-2
    best = (0, 0)
    for a in range(0, 7):
        for b in range(0, min(a, 6) + 1, 2):
            err = np.sqrt(BETA * BETA * 2.0 * (E(a) + E(b)) + BFERR * BFERR)
            if err <= THR and a + b > best[0] + best[1]:
                best = (a, b)
    ns81, ns82 = best
    has_add = bool(np.asarray(U_additional).any()) and bool(
        np.asarray(Vh_additional).any()
    )
    extra = A if has_add else 0
    NBF1 = (R - ns81 * 128 + extra + 127) // 128
    return NBF1, ns81, ns82, order, has_add


def _prep_maps(input, weight, U, Vh, U_additional, Vh_additional, mask, sel):
    import ml_dtypes

    E4 = ml_dtypes.float8_e4m3
    BF = ml_dtypes.bfloat16
    NBF1, NS81, NS82, order, has_add, names = sel
    NS1 = NS81 * 128
    NS2 = NS82 * 128
    NMID = NS81 - NS82
    NBF2 = NBF1 + NMID
    NF1 = NBF1 * 128
    NF2 = NBF2 * 128

    s = (weight * weight * mask).astype(np.float32)

    def q8(a, sc):
        return np.clip(a * sc, -240.0, 240.0).astype(E4)

    # ranks permuted s-ascending; fp8 sets are prefixes of that layout
    Vp = Vh[order]                       # [R, D_IN]
    Up = U[:, order] * s[order][None, :]  # [D_OUT, R]

    # ---- shared (per-core-identical) operands ----
    # phase-1 bf16 V: ranks NS1.. (+ additional + zero pad)
    VF = np.zeros((NF1, D_IN), np.float32)
    VF[: R - NS1] = Vp[NS1:]
    if has_add:
        VF[R - NS1 : R - NS1 + A] = Vh_additional
    vbf = np.ascontiguousarray(VF.T.reshape(KT, 128, NF1).astype(BF))
    # phase-2 bf16 U: ranks NS2.. (pre-scaled by C), additional, pad
    UF = np.zeros((D_OUT, NF2), np.float32)
    UF[:, : R - NS2] = Up[:, NS2:] * C_SCALE
    if has_add:
        UF[:, R - NS2 : R - NS2 + A] = U_additional * C_SCALE
    ubf = np.ascontiguousarray(
        UF.T.reshape(NBF2, 128, D_OUT).transpose(1, 0, 2).astype(BF)
    )
    shared = {names["vbf"]: vbf, names["ubf"]: ubf}
    if NS81:
        v8 = q8(Vp[:NS1].T, AV).reshape(KP, 2, 128, NS1).transpose(0, 2, 1, 3)
        shared[names["v8"]] = np.ascontiguousarray(v8)
        NP8 = NS82 // 2
        u8 = (
            q8(Up[:, :NS2].T, AU)
            .reshape(NP8, 2, 128, D_OUT)
            .transpose(2, 0, 1, 3)
        )
        shared[names["u8"]] = np.ascontiguousarray(u8)

    x2 = np.asarray(input, dtype=np.float32).reshape(T, D_IN)
    in_maps = []
    for c in range(N_CORES):
        xcT = np.ascontiguousarray(x2[c * TC : (c + 1) * TC].T)
        m = dict(shared)
        m[names["xbf"]] = np.ascontiguousarray(xcT.reshape(KT, 128, TC).astype(BF))
        if NS81 and "x8" in names:
            m[names["x8"]] = np.ascontiguousarray(
                q8(xcT, AX).reshape(KP, 2, 128, TC).transpose(0, 2, 1, 3)
            )
        in_maps.append(m)
    return in_maps


def _gather(results, yname):
    out = np.empty((T, D_OUT), np.float32)
    for c in range(N_CORES):
        out[c * TC : (c + 1) * TC] = results[c][yname].T
    return out.reshape(B, S, D_OUT)


def kernel(input, weight, U, Vh, U_additional, Vh_additional, mask, **_kw):
    from concourse.bass_utils import run_bass_kernel_spmd

    input = np.asarray(input, dtype=np.float32)
    weight = np.asarray(weight, dtype=np.float32)
    U = np.asarray(U, dtype=np.float32)
    Vh = np.asarray(Vh, dtype=np.float32)
    U_additional = np.asarray(U_additional, dtype=np.float32)
    Vh_additional = np.asarray(Vh_additional, dtype=np.float32)
    mask = np.asarray(mask, dtype=np.float32)

    sel = _select(weight, mask, U_additional, Vh_additional)
    # on-device bf16->fp8 cast measured slower (GpSimd casts serialize with
    # the x-tile ring); host-prepared x8 via DMA wins
    nc, names = _build(sel[0], sel[1], sel[2])
    sel = sel + (names,)
    in_maps = _prep_maps(
        input, weight, U, Vh, U_additional, Vh_additional, mask, sel
    )
    res = run_bass_kernel_spmd(nc, in_maps, core_ids=list(range(N_CORES)))
    return _gather(res.results, names["yT"])



# revision 5
# speedup vs baseline: 1.0533x; 1.0533x over previous
"""TRN2 Bass kernel for nn_BSLinear_71159018160311.

Computes  out = input @ W.T  with
  W = U @ diag(weight^2 * mask) @ Vh + U_additional @ Vh_additional

Sharding: data-parallel over the B*S=16384 token dim across 8 NeuronCores
(2048 tokens/core), no collectives. Each core runs the factorized form
(t = V_eff @ x, then y = U_eff @ t) as two matmul phases.

Mixed-precision rank split: the rank-r component of W contributes with
weight s_r = weight_r^2 * mask_r, so fp8-ing a rank subset with s^2-energy
share E costs rel err ~ beta*sqrt(2E) per phase (beta ~ 2.7% for e4m3 on
Gaussian data). The two phases have independent budgets: phase 1 runs the
640 lowest-s ranks in fp8 e4m3 DoubleRow (2 k-tiles contracted per PE
pass - 2x rate), phase 2 the lowest 512 (rank-pairs must be even); the
128 mid ranks are cast from the fp8 psum to true-scale bf16 and join the
bf16 side of phase 2. Everything else runs bf16 (full PE rate, ~0.1%
error). Measured end-to-end rel err 1.60e-2 vs the 2e-2 gate.

Scales (all powers of two, so rescaling is exact and program immediates are
data-independent): x*32 -> fp8, V*2048 -> fp8, U_eff*4096 -> fp8, and the
phase-1 psum (scale 32*2048) is cast to the phase-2 fp8 operand t8 with a
single tensor_scalar_mul by 2^-12 (target scale 16). The bf16 branch's U is
pre-scaled by C=16*4096=2^16 so both branches accumulate in the SAME psum
group; the output copy multiplies by 2^-16.

Phase 1, bf16 half: token-halved full-K psum - all (<=4) bf16 rank-tile
psum groups ([128,1024], 2 banks each) stay open across the whole K=4096
contraction for one token-half, so there are no SBUF accumulation adds at
all; the second token-half reuses the x ring slots. The fp8 half keeps x8
(8MB) + v8 resident in SBUF (their loads queue behind both x token-half
streams and land just-in-time), accumulates full-K in psum, then one
scaled cast per half-group emits t8 in the [128, 2, TC] k-pair layout
DoubleRow wants (or true-scale bf16 for the mid tiles). Phase 2
accumulates bf16 rank-tiles then fp8 rank-pairs into one psum group per
128-row dout tile (u chunks prefetched one 512-dout chunk ahead of the
y-store DMAs), scaled-copies to SBUF on alternating DVE/ACT and DMAs out;
the very last group is split 1024/512/512 to shorten the drain tail.

When U_additional/Vh_additional are nonzero (they are zero for this
problem instance) the A=64 extra ranks join the bf16 half (zero-padded to
a full 128 tile).
"""

import functools

import numpy as np

B, S, D_IN, D_OUT, R, A = 4, 4096, 4096, 4096, 1024, 64
N_CORES = 8
T = B * S
TC = T // N_CORES  # 2048
KT = D_IN // 128  # 32 k-tiles
KP = KT // 2  # 16 k-pairs (fp8 DoubleRow)
KB = 4  # bf16 k-tiles per stream block
NB = KT // KB  # 8 blocks
NC_OUT = D_OUT // 512  # 8 chunks of 512 dout rows

# power-of-two scales (exact rescaling, data-independent immediates)
AX, AV, AU, AT = 32.0, 2048.0, 4096.0, 16.0
C_SCALE = AT * AU  # 65536
C1 = AT / (AX * AV)  # 2^-12: psum(phase1 fp8) -> t8
INV_C = 1.0 / C_SCALE  # 2^-16: psum(phase2) -> y


@functools.lru_cache(maxsize=4)
def _build(NBF1, NS81, NS82, devcast=False):
    """NBF1: phase-1 bf16 rank-tiles; NS81: phase-1 fp8 tiles; NS82: phase-2
    fp8 tiles (even, subset of the phase-1 fp8 set). Ranks are laid out in
    s-ascending order; tiles [NS82, NS81) are fp8 in phase 1 but bf16 in
    phase 2 (their t is cast to bf16 at true scale)."""
    import concourse.bacc as bacc
    import concourse.mybir as mybir
    import concourse.tile as tile

    NP8 = (NS82 + 1) // 2  # phase-2 fp8 rank-pair tiles (last may be padded)
    NMID = NS81 - NS82  # fp8-phase1 / bf16-phase2 tiles
    NBF2 = NBF1 + NMID  # phase-2 bf16 rank-tiles
    NF = NBF1 * 128
    NS = NS81 * 128
    f8 = mybir.dt.float8e4
    bf16 = mybir.dt.bfloat16
    f32 = mybir.dt.float32
    add = mybir.AluOpType.add
    ACT_COPY = mybir.ActivationFunctionType.Copy
    DR = mybir.MatmulPerfMode.DoubleRow
    H = TC // 2  # 1024: psum half-group token width
    CMID = 1.0 / (AX * AV)  # 2^-16: phase-1 fp8 psum -> true-scale bf16 t

    nc = bacc.Bacc(trn_type="TRN2")
    with tile.TileContext(nc) as tc:
        with tc.tile_pool(name="dram", bufs=1, space="DRAM") as dram:
            xbf = dram.tile([KT, 128, TC], bf16, kind="ExternalInput", name="xbf")
            vbf = dram.tile([KT, 128, NF], bf16, kind="ExternalInput", name="vbf")
            ubf = dram.tile([128, NBF2, D_OUT], bf16, kind="ExternalInput", name="ubf")
            if NS81:
                if not devcast:
                    x8d = dram.tile(
                        [KP, 128, 2, TC], f8, kind="ExternalInput", name="x8"
                    )
                v8d = dram.tile([KP, 128, 2, NS], f8, kind="ExternalInput", name="v8")
                u8d = dram.tile(
                    [128, NP8, 2, D_OUT], f8, kind="ExternalInput", name="u8"
                )
            yT = dram.tile([D_OUT, TC], f32, kind="ExternalOutput", name="yT")

            with (
                tc.tile_pool(name="tbf", bufs=NBF2) as tbfpool,
                tc.tile_pool(name="t8", bufs=max(NP8, 1)) as t8pool,
                tc.tile_pool(name="x8r", bufs=max(KP, 1)) as x8pool,
                tc.tile_pool(name="v8r", bufs=max(KP, 1)) as v8pool,
                tc.tile_pool(name="u0", bufs=1) as u0pool,
                tc.tile_pool(name="ps", bufs=4, space="PSUM") as pspool,
            ):
                t_bf = [tbfpool.tile([128, TC], bf16, name="tbf") for _ in range(NBF2)]
                t8 = [t8pool.tile([128, 2, TC], f8, name="t8") for _ in range(NP8)]
                if NS82 % 2:
                    # odd fp8 tile count: the last pair's second k-row is
                    # padding — u8 has zeros there, but 0 * uninit-SBUF can
                    # still be NaN in the PE accumulation, so zero t8 too
                    nc.vector.memset(t8[NP8 - 1][:, 1, :], 0.0)
                x8_t = [
                    x8pool.tile([128, 2, TC], f8, name="x8r")
                    for _ in range(KP if NS81 else 0)
                ]
                v8_t = [
                    v8pool.tile([128, 2, NS], f8, name="v8r")
                    for _ in range(KP if NS81 else 0)
                ]
                # phase-2 chunk-0 weights: loaded in background during phase 1
                u0bf = u0pool.tile([128, NBF2, 512], bf16)
                u08 = u0pool.tile([128, max(NP8, 1), 2, 512], f8)

                # ---- phase 1: bf16 half (token-halved full-K psum) ----
                # all NBF1<=4 rank-tile psum groups (2 banks each) stay open
                # across the whole K=4096 contraction for one token-half, so
                # the bf16 half needs NO SBUF accumulation adds at all; the
                # second token-half reuses the x ring slots (WAR-linked).
                with (
                    tc.tile_pool(name="xk", bufs=KT) as xpool,
                    tc.tile_pool(name="vk", bufs=KT) as vpool,
                ):
                    vts = [
                        vpool.tile([128, NF], bf16, name="vk") for _ in range(KT)
                    ]
                    MCH = min(NBF1, 4)
                    passes = [
                        (list(range(c, min(c + MCH, NBF1))), h)
                        for c in range(0, NBF1, MCH)
                        for h in range(2)
                    ]

                    def issue_pass(pi):
                        ch, h = passes[pi]
                        xts = []
                        for k in range(KT):
                            t = xpool.tile([128, H], bf16, name="xk")
                            if pi == 0:
                                nc.sync.dma_start(vts[k][:], vbf[k])
                            if pi == 0 and k == 0:
                                # startup: 512-col slices so the first matmul
                                # fires as soon as slice 0 lands
                                for q in range(2):
                                    nc.sync.dma_start(
                                        t[:, q * 512 : (q + 1) * 512],
                                        xbf[0][:, q * 512 : (q + 1) * 512],
                                    )
                            else:
                                nc.sync.dma_start(
                                    t[:], xbf[k][:, h * H : (h + 1) * H]
                                )
                            xts.append(t)
                        return xts

                    pending = {0: issue_pass(0)}
                    if len(passes) > 1:
                        pending[1] = issue_pass(1)
                    # resident fp8 operands + phase-2 chunk-0 weights queue
                    # behind both token-half streams; the fp8 phase consumes
                    # pairs in order just as they land
                    if NS81:
                        for p in range(KP):
                            nc.sync.dma_start(x8_t[p][:], x8d[p])
                            nc.sync.dma_start(v8_t[p][:], v8d[p])
                    nc.sync.dma_start(u0bf[:], ubf[:, :, 0:512])
                    if NS82:
                        nc.sync.dma_start(u08[:], u8d[:, :, :, 0:512])

                    for pi, (ch, h) in enumerate(passes):
                        xts = pending.pop(pi)
                        if pi + 2 < len(passes) and pi + 2 not in pending:
                            pending[pi + 2] = issue_pass(pi + 2)
                        psums = [
                            pspool.tile([128, H], f32, name="ps") for _ in ch
                        ]
                        for k in range(KT):
                            for mi, m in enumerate(ch):
                                for n2 in range(2):
                                    nc.tensor.matmul(
                                        psums[mi][:, n2 * 512 : (n2 + 1) * 512],
                                        lhsT=vts[k][:, m * 128 : (m + 1) * 128],
                                        rhs=xts[k][:, n2 * 512 : (n2 + 1) * 512],
                                        start=(k == 0),
                                        stop=(k == KT - 1),
                                    )
                        for mi, m in enumerate(ch):
                            nc.any.tensor_copy(
                                t_bf[NMID + m][:, h * H : (h + 1) * H],
                                psums[mi][:, :],
                            )

                # ---- phase 1: fp8 half (x8 resident, full-K psum) ----
                # mid tiles (bf16 in phase 2) first: phase-2's first bf16
                # matmuls read them, so their casts must land earliest
                for m8 in list(range(NS82, NS81)) + list(range(NS82)):
                    for h in range(2):
                        psum = pspool.tile([128, H], f32, name="ps")
                        for p in range(KP):
                            for ni in range(2):
                                n = 2 * h + ni
                                nc.tensor.matmul(
                                    psum[:, ni * 512 : (ni + 1) * 512],
                                    lhsT=v8_t[p][:, :, m8 * 128 : (m8 + 1) * 128],
                                    rhs=x8_t[p][:, :, n * 512 : (n + 1) * 512],
                                    start=(p == 0),
                                    stop=(p == KP - 1),
                                    perf_mode=DR,
                                )
                        if m8 < NS82:
                            nc.any.tensor_scalar_mul(
                                t8[m8 // 2][:, m8 % 2, h * H : (h + 1) * H],
                                psum[:, :],
                                C1,
                            )
                        else:
                            nc.any.tensor_scalar_mul(
                                t_bf[m8 - NS82][:, h * H : (h + 1) * H],
                                psum[:, :],
                                CMID,
                            )

                # ---- phase 2 ----
                with (
                    tc.tile_pool(name="ud", bufs=2) as upool,
                    tc.tile_pool(name="ysb", bufs=4) as ypool,
                ):
                    u_cur = (u0bf, u08)
                    u_next = None
                    for ci in range(NC_OUT):
                        ubf_t, u8_t = u_cur
                        if ci + 1 < NC_OUT:
                            # prefetch next chunk's weights ahead of this
                            # chunk's y-store DMAs so the PE never waits
                            nci = ci + 1
                            nbf = upool.tile([128, NBF2, 512], bf16, name="udb")
                            nc.sync.dma_start(
                                nbf[:], ubf[:, :, nci * 512 : (nci + 1) * 512]
                            )
                            n8 = None
                            if NS82:
                                n8 = upool.tile([128, NP8, 2, 512], f8, name="ud8")
                                nc.sync.dma_start(
                                    n8[:], u8d[:, :, :, nci * 512 : (nci + 1) * 512]
                                )
                            u_next = (nbf, n8)
                        for dd in range(4):
                            row = ci * 512 + dd * 128
                            # the very last half-group is split into 512-wide
                            # quarters so the post-matmul copy+store tail is
                            # as short as possible
                            final = ci == NC_OUT - 1 and dd == 3
                            pieces = [(0, (0, 1)), (1, (2,)), (2, (3,))] if final else [
                                (0, (0, 1)), (1, (2, 3))
                            ]
                            for pi, (slot, ns_) in enumerate(pieces):
                                W = len(ns_) * 512
                                psum = pspool.tile([128, W], f32, name="ps")
                                for kt in range(NBF2):
                                    for ni, n in enumerate(ns_):
                                        nc.tensor.matmul(
                                            psum[:, ni * 512 : (ni + 1) * 512],
                                            lhsT=ubf_t[:, kt, dd * 128 : (dd + 1) * 128],
                                            rhs=t_bf[kt][:, n * 512 : (n + 1) * 512],
                                            start=(kt == 0),
                                            stop=(NS82 == 0 and kt == NBF2 - 1),
                                        )
                                for pt in range(NP8):
                                    for ni, n in enumerate(ns_):
                                        nc.tensor.matmul(
                                            psum[:, ni * 512 : (ni + 1) * 512],
                                            lhsT=u8_t[:, pt, :, dd * 128 : (dd + 1) * 128],
                                            rhs=t8[pt][:, :, n * 512 : (n + 1) * 512],
                                            start=False,
                                            stop=(pt == NP8 - 1),
                                            perf_mode=DR,
                                        )
                                ysb = ypool.tile([128, W], f32, name="ysb")
                                if pi == 1:
                                    # ACT: out = Copy(in * scale); GpSimd
                                    # cannot read PSUM on hardware
                                    nc.scalar.activation(
                                        ysb[:], psum[:, :], ACT_COPY, scale=INV_C
                                    )
                                else:
                                    nc.vector.tensor_scalar_mul(
                                        ysb[:], psum[:, :], INV_C
                                    )
                                col = ns_[0] * 512
                                nc.sync.dma_start(
                                    yT[row : row + 128, col : col + W],
                                    ysb[:],
                                )
                        u_cur = u_next
    nc.compile()
    names = {"xbf": xbf.name, "vbf": vbf.name, "ubf": ubf.name, "yT": yT.name}
    if NS81:
        names.update({"v8": v8d.name, "u8": u8d.name})
        if not devcast:
            names["x8"] = x8d.name
    return nc, names


def _select(weight, mask, U_additional, Vh_additional):
    """Pick (NBF1, NS81, NS82) from the weights: the largest fp8 coverage
    whose estimated error keeps margin inside the 2e-2 gate. Phase-1 fp8
    covers tiles [0, NS81), phase-2 fp8 tiles [0, NS82) (even), ranks laid
    out s-ascending."""
    s = (weight * weight * mask).astype(np.float32)
    order = np.argsort(s, kind="stable")
    s2 = np.sort(s.astype(np.float64) ** 2)
    tot = float(np.sum(s2)) or 1.0
    cum = np.cumsum(s2) / tot

    def E(tiles):
        n = tiles * 128
        return float(cum[n - 1]) if n else 0.0

    # BETA calibrated against the measured end-to-end rel err of the (5,4)
    # config (1.575e-2 vs model 1.72e-2 at BETA=0.027). THR leaves ~0.07e-2
    # of the 2e-2 gate as cross-machine margin for the predicted (5,5) pick
    # (est 1.86e-2).
    BETA, BFERR, THR = 0.0246, 4.5e-3, 1.93e-2
    best, best_cost = (0, 0), None
    for a in range(0, 7):
        for b in range(0, min(a, 6) + 1):
            err = np.sqrt(BETA * BETA * 2.0 * (E(a) + E(b)) + BFERR * BFERR)
            # PE cost in 16384-cycle units: bf16 tile = 4, phase-1 fp8
            # tile = 1, phase-2 fp8 pair = 2 (odd b pads the last pair)
            cost = (8 - a) * 4 + a + (8 - b) * 4 + 2 * ((b + 1) // 2)
            if err <= THR and (best_cost is None or cost < best_cost):
                best, best_cost = (a, b), cost
    ns81, ns82 = best
    has_add = bool(np.asarray(U_additional).any()) and bool(
        np.asarray(Vh_additional).any()
    )
    extra = A if has_add else 0
    NBF1 = (R - ns81 * 128 + extra + 127) // 128
    return NBF1, ns81, ns82, order, has_add


def _prep_maps(input, weight, U, Vh, U_additional, Vh_additional, mask, sel):
    import ml_dtypes

    E4 = ml_dtypes.float8_e4m3
    BF = ml_dtypes.bfloat16
    NBF1, NS81, NS82, order, has_add, names = sel
    NS1 = NS81 * 128
    NS2 = NS82 * 128
    NMID = NS81 - NS82
    NBF2 = NBF1 + NMID
    NF1 = NBF1 * 128
    NF2 = NBF2 * 128

    s = (weight * weight * mask).astype(np.float32)

    def q8(a, sc):
        return np.clip(a * sc, -240.0, 240.0).astype(E4)

    # ranks permuted s-ascending; fp8 sets are prefixes of that layout
    Vp = Vh[order]                       # [R, D_IN]
    Up = U[:, order] * s[order][None, :]  # [D_OUT, R]

    # ---- shared (per-core-identical) operands ----
    # phase-1 bf16 V: ranks NS1.. (+ additional + zero pad)
    VF = np.zeros((NF1, D_IN), np.float32)
    VF[: R - NS1] = Vp[NS1:]
    if has_add:
        VF[R - NS1 : R - NS1 + A] = Vh_additional
    vbf = np.ascontiguousarray(VF.T.reshape(KT, 128, NF1).astype(BF))
    # phase-2 bf16 U: ranks NS2.. (pre-scaled by C), additional, pad
    UF = np.zeros((D_OUT, NF2), np.float32)
    UF[:, : R - NS2] = Up[:, NS2:] * C_SCALE
    if has_add:
        UF[:, R - NS2 : R - NS2 + A] = U_additional * C_SCALE
    ubf = np.ascontiguousarray(
        UF.T.reshape(NBF2, 128, D_OUT).transpose(1, 0, 2).astype(BF)
    )
    shared = {names["vbf"]: vbf, names["ubf"]: ubf}
    if NS81:
        v8 = q8(Vp[:NS1].T, AV).reshape(KP, 2, 128, NS1).transpose(0, 2, 1, 3)
        shared[names["v8"]] = np.ascontiguousarray(v8)
        NP8 = (NS82 + 1) // 2
        Upq = np.zeros((NP8 * 256, D_OUT), np.float32)
        Upq[:NS2] = Up[:, :NS2].T
        u8 = q8(Upq, AU).reshape(NP8, 2, 128, D_OUT).transpose(2, 0, 1, 3)
        shared[names["u8"]] = np.ascontiguousarray(u8)

    x2 = np.asarray(input, dtype=np.float32).reshape(T, D_IN)
    in_maps = []
    for c in range(N_CORES):
        xcT = np.ascontiguousarray(x2[c * TC : (c + 1) * TC].T)
        m = dict(shared)
        m[names["xbf"]] = np.ascontiguousarray(xcT.reshape(KT, 128, TC).astype(BF))
        if NS81 and "x8" in names:
            m[names["x8"]] = np.ascontiguousarray(
                q8(xcT, AX).reshape(KP, 2, 128, TC).transpose(0, 2, 1, 3)
            )
        in_maps.append(m)
    return in_maps


def _gather(results, yname):
    out = np.empty((T, D_OUT), np.float32)
    for c in range(N_CORES):
        out[c * TC : (c + 1) * TC] = results[c][yname].T
    return out.reshape(B, S, D_OUT)


def kernel(input, weight, U, Vh, U_additional, Vh_additional, mask, **_kw):
    from concourse.bass_utils import run_bass_kernel_spmd

    input = np.asarray(input, dtype=np.float32)
    weight = np.asarray(weight, dtype=np.float32)
    U = np.asarray(U, dtype=np.float32)
    Vh = np.asarray(Vh, dtype=np.float32)
    U_additional = np.asarray(U_additional, dtype=np.float32)
    Vh_additional = np.asarray(Vh_additional, dtype=np.float32)
    mask = np.asarray(mask, dtype=np.float32)

    sel = _select(weight, mask, U_additional, Vh_additional)
    # on-device bf16->fp8 cast measured slower (GpSimd casts serialize with
    # the x-tile ring); host-prepared x8 via DMA wins
    nc, names = _build(sel[0], sel[1], sel[2])
    sel = sel + (names,)
    in_maps = _prep_maps(
        input, weight, U, Vh, U_additional, Vh_additional, mask, sel
    )
    res = run_bass_kernel_spmd(nc, in_maps, core_ids=list(range(N_CORES)))
    return _gather(res.results, names["yT"])

